# revision 1
# baseline (speedup 1.0000x reference)
"""Trainium2 Bass kernel for nn_BWCaster_86337432584570 (embedding_lookup).

sigma[n,j] = relu( sum_p sum_c bilinear(plane[j,p])[c](x,y) * linear(line[j,p])[c](z) )

Device algorithm (per NeuronCore, points sharded 8 ways):
  - bf16 chunk tables in HBM: per (j,p) plane rows [16c,2y,4x] (256B) covering
    corner pairs, and line rows [16c,8z] (256B) covering z-pairs.
  - gpsimd dma_gather fetches 6 chunks per (point, j): 3 plane + 3 line.
  - DVE multiplies chunks by host-packed bf16 weights (2-sparse tents),
    reduces, multiplies plane*line factors and reduces over (p, c).
Host does layout prep: table expansion, coordinate transform, index/weight
packing (the per-point lookup+interpolation — the dominant work — runs on HW).
"""
import sys
import numpy as np
import ml_dtypes

sys.path.insert(0, "/opt/trn_rl_repo")

import concourse.bass as bass
import concourse.bacc as bacc
import concourse.mybir as mybir
from concourse.bass_utils import run_bass_kernel_spmd
from concourse.library_config import mlp

# ---------------- problem constants (hardcoded) ----------------
N_TOTAL, J, C, G = 262144, 24, 16, 128
N_CORES = 8
NPTS = N_TOTAL // N_CORES          # 32768 points per core
SB = 2048                          # points per superblock
NSB = NPTS // SB                   # 16
NB = SB // 128                     # 16 sub-blocks of 128 points
NSLOT = 5 * NB                     # 80 gather slots per (j, superblock)
NIDX = NSLOT * 128                 # 10240 indices per (j, superblock)
GSPLIT = 1                         # gather sub-calls per superblock
GIDX = NIDX // GSPLIT              # indices per dma_gather call
GSLOT = NSLOT // GSPLIT
NPLANE_ROWS = 127 * 63             # 8001
NPAIR_ROWS = 63 * 63               # 3969 (line0,line1) window combos
NLINE_ROWS = 31
TABROWS = 3 * NPLANE_ROWS + NPAIR_ROWS + NLINE_ROWS  # 28003
W8LEN = 3 * NB * 8 + NB * 2 * 4 + NB * 8  # 640
MAT_MODE = [(0, 1), (0, 2), (1, 2)]
VEC_MODE = [2, 1, 0]
BF16 = mybir.dt.bfloat16
F32 = mybir.dt.float32
I16 = mybir.dt.int16

_CACHE = {}


# ---------------- host-side prep ----------------
def _build_tables(planes, lines):
    tab = np.zeros((J, TABROWS, 128), dtype=np.float32)
    for p in range(3):
        sw = np.lib.stride_tricks.sliding_window_view(planes[p], (2, 4), axis=(2, 3))
        sw = sw[:, :, :127, 0:125:2]                       # [J,C,127,63,2,4]
        ep = sw.transpose(0, 2, 3, 1, 4, 5).reshape(J, NPLANE_ROWS, 128)
        tab[:, p * NPLANE_ROWS:(p + 1) * NPLANE_ROWS, :] = ep
    # pair table: rows (zw0, zw1) -> [2, 16c, 4z] from line0/line1 windows of 4
    w0 = np.lib.stride_tricks.sliding_window_view(lines[0], 4, axis=2)[:, :, 0:125:2]
    w1 = np.lib.stride_tricks.sliding_window_view(lines[1], 4, axis=2)[:, :, 0:125:2]
    # w0/w1: [J, C, 63, 4] -> combos [J, 63, 63, 2, C, 4]
    pair = np.empty((J, 63, 63, 2, C, 4), np.float32)
    pair[:, :, :, 0, :, :] = w0.transpose(0, 2, 1, 3)[:, :, None, :, :]
    pair[:, :, :, 1, :, :] = w1.transpose(0, 2, 1, 3)[:, None, :, :, :]
    tab[:, 3 * NPLANE_ROWS:3 * NPLANE_ROWS + NPAIR_ROWS, :] = \
        pair.reshape(J, NPAIR_ROWS, 128)
    # line2 table: [16c, 8z] windows of 8 at stride 4
    swl = np.lib.stride_tricks.sliding_window_view(lines[2], 8, axis=2)
    swl = swl[:, :, 0:121:4]                               # [J,C,31,8]
    el = swl.transpose(0, 2, 1, 3).reshape(J, NLINE_ROWS, 128)
    base = 3 * NPLANE_ROWS + NPAIR_ROWS
    tab[:, base:base + NLINE_ROWS, :] = el
    return tab.astype(ml_dtypes.bfloat16)


def _coords_weights(xyz, transforms):
    N = xyz.shape[0]
    xyzh = np.concatenate([xyz, np.ones((N, 1), np.float32)], axis=1)
    pts = np.einsum('jab,nb->nja', transforms[:, :3, :].astype(np.float32), xyzh)
    coord = (pts * np.float32(2.0 / 3.0) + np.float32(1.0)) * np.float32(0.5 * (G - 1))
    c0 = np.floor(coord).astype(np.int32)
    fr = (coord - c0).astype(np.float32)

    idx5 = np.zeros((N, J, 5), np.int16)
    w8p = np.zeros((N, J, 3, 8), np.float32)
    wpair = np.zeros((N, J, 2, 4), np.float32)
    wl2 = np.zeros((N, J, 8), np.float32)
    zws = []
    for p in range(3):
        m0, m1 = MAT_MODE[p]
        v = VEC_MODE[p]
        x0, y0, z0 = c0[:, :, m0], c0[:, :, m1], c0[:, :, v]
        fx, fy, fz = fr[:, :, m0], fr[:, :, m1], fr[:, :, v]
        xw = np.minimum(x0 >> 1, 62)
        ox = x0 - 2 * xw
        idx5[:, :, p] = (p * NPLANE_ROWS + y0 * 63 + xw).astype(np.int16)
        wx4 = np.zeros((N, J, 4), np.float32)
        np.put_along_axis(wx4, ox[..., None], (1.0 - fx)[..., None], axis=2)
        np.put_along_axis(wx4, (ox + 1)[..., None], fx[..., None], axis=2)
        w8p[:, :, p, 0:4] = (1.0 - fy)[..., None] * wx4
        w8p[:, :, p, 4:8] = fy[..., None] * wx4
        if p < 2:  # lines 0,1: window-4 combo
            zw = np.minimum(z0 >> 1, 62)
            oz = z0 - 2 * zw
            zws.append(zw)
            wz4 = np.zeros((N, J, 4), np.float32)
            np.put_along_axis(wz4, oz[..., None], (1.0 - fz)[..., None], axis=2)
            np.put_along_axis(wz4, (oz + 1)[..., None], fz[..., None], axis=2)
            wpair[:, :, p, :] = wz4
        else:  # line2: window-8
            zw = np.minimum(z0 >> 2, 30)
            oz = z0 - 4 * zw
            idx5[:, :, 4] = (3 * NPLANE_ROWS + NPAIR_ROWS + zw).astype(np.int16)
            wz8 = np.zeros((N, J, 8), np.float32)
            np.put_along_axis(wz8, oz[..., None], (1.0 - fz)[..., None], axis=2)
            np.put_along_axis(wz8, (oz + 1)[..., None], fz[..., None], axis=2)
            wl2[:, :, :] = wz8
    idx5[:, :, 3] = (3 * NPLANE_ROWS + zws[0] * 63 + zws[1]).astype(np.int16)
    return idx5, w8p, wpair, wl2


def _pack_core(idx5, w8p, wpair, wl2):
    """Per-core packing.
    idx5 [NPTS,J,5] -> idx dram [J, NSB, 128, NIDX//16] int16 (wrapped+replicated)
    weights -> w8 dram [J, NSB, 128, W8LEN] bf16
      layout per np: [3p,16nb,8r | 16nb,2l,4r | 16nb,8r]
    """
    u = idx5.reshape(NSB, NB, 128, J, 5)
    arr = u.transpose(3, 0, 4, 1, 2).reshape(J, NSB, NSLOT * 128)
    wrapped = arr.reshape(J, NSB, NIDX // 16, 16).transpose(0, 1, 3, 2)
    idx_dram = np.broadcast_to(wrapped[:, :, None, :, :], (J, NSB, 8, 16, NIDX // 16))
    idx_dram = np.ascontiguousarray(idx_dram).reshape(J, NSB, 128, NIDX // 16)

    wp = w8p.reshape(NSB, NB, 128, J, 3, 8).transpose(3, 0, 2, 4, 1, 5)  # [j,sb,np,p,nb,r]
    wq = wpair.reshape(NSB, NB, 128, J, 2, 4).transpose(3, 0, 2, 1, 4, 5)  # [j,sb,np,nb,l,r]
    w2 = wl2.reshape(NSB, NB, 128, J, 8).transpose(3, 0, 2, 1, 4)  # [j,sb,np,nb,r]
    w8_dram = np.concatenate([
        np.ascontiguousarray(wp).reshape(J, NSB, 128, 384),
        np.ascontiguousarray(wq).reshape(J, NSB, 128, 128),
        np.ascontiguousarray(w2).reshape(J, NSB, 128, 128),
    ], axis=3).astype(ml_dtypes.bfloat16)
    return idx_dram, w8_dram


# ---------------- device kernel ----------------
def _build_bass(nit_lim=None):
    nc = bacc.Bacc("TRN2")
    tab = nc.dram_tensor("tab", [J, TABROWS, 128], BF16, kind="ExternalInput")
    idx = nc.dram_tensor("idx", [J, NSB, 128, NIDX // 16], I16, kind="ExternalInput")
    w8 = nc.dram_tensor("w8", [J, NSB, 128, W8LEN], BF16, kind="ExternalInput")
    # device-native layout [np, sb, nb, j]; host transposes to [NPTS, J]
    out = nc.dram_tensor("out", [128, NSB, NB, J], F32, kind="ExternalOutput")

    NIT = J * NSB  # 384 iterations, j outer / sb inner
    if nit_lim is not None:
        NIT = nit_lim

    from contextlib import ExitStack
    with ExitStack() as ctx:
        dst = ctx.enter_context(nc.sbuf_tensor("dst", [128, 2, NSLOT, 128], BF16))
        idxs = ctx.enter_context(nc.sbuf_tensor("idxs", [128, 2, NIDX // 16], I16))
        w8t = ctx.enter_context(nc.sbuf_tensor("w8t", [128, 2, W8LEN], BF16))
        wprod = ctx.enter_context(nc.sbuf_tensor("wprod", [128, 6144], BF16))
        t1 = ctx.enter_context(nc.sbuf_tensor("t1", [128, 3072], BF16))
        t2 = ctx.enter_context(nc.sbuf_tensor("t2", [128, 1536], BF16))
        pf = ctx.enter_context(nc.sbuf_tensor("pf", [128, 768], F32))
        wq = ctx.enter_context(nc.sbuf_tensor("wq", [128, 2048], BF16))
        wq1 = ctx.enter_context(nc.sbuf_tensor("wq1", [128, 1024], BF16))
        lf01 = ctx.enter_context(nc.sbuf_tensor("lf01", [128, 512], F32))
        w2 = ctx.enter_context(nc.sbuf_tensor("w2", [128, 2048], BF16))
        w21 = ctx.enter_context(nc.sbuf_tensor("w21", [128, 1024], BF16))
        w22 = ctx.enter_context(nc.sbuf_tensor("w22", [128, 512], BF16))
        lf2 = ctx.enter_context(nc.sbuf_tensor("lf2", [128, 256], F32))
        prod = ctx.enter_context(nc.sbuf_tensor("prod", [128, 768], F32))
        outt = ctx.enter_context(nc.sbuf_tensor("outt", [128, NSB, NB, J], F32))
        s_idx0 = ctx.enter_context(nc.semaphore("s_idx0"))
        s_idx1 = ctx.enter_context(nc.semaphore("s_idx1"))
        s_gat0 = ctx.enter_context(nc.semaphore("s_gat0"))
        s_gat1 = ctx.enter_context(nc.semaphore("s_gat1"))
        s_w80 = ctx.enter_context(nc.semaphore("s_w80"))
        s_w81 = ctx.enter_context(nc.semaphore("s_w81"))
        s_cmb = ctx.enter_context(nc.semaphore("s_cmb"))
        s_relu = ctx.enter_context(nc.semaphore("s_relu"))
        s_out = ctx.enter_context(nc.semaphore("s_out"))
        s_v = ctx.enter_context(nc.semaphore("s_v"))
        block = ctx.enter_context(nc.Block())
        s_gat = [s_gat0, s_gat1]
        s_w8 = [s_w80, s_w81]
        s_idx = [s_idx0, s_idx1]

        @block.gpsimd
        def _(gpsimd):
            gpsimd.load_library(mlp)
            for it in range(NIT):
                j, sb = it // NSB, it % NSB
                b = it % 2
                if it >= 2:
                    gpsimd.wait_ge(s_cmb, it - 1)
                gpsimd.wait_ge(s_idx[b], 16 * (it // 2 + 1))
                for g in range(GSPLIT):
                    gpsimd.dma_gather(
                        dst[:, b, g * GSLOT:(g + 1) * GSLOT, :], tab[j],
                        idxs[:, b, g * (GIDX // 16):(g + 1) * (GIDX // 16)],
                        GIDX, GIDX, 128, single_packet=False,
                    ).then_inc(s_gat[b], 16)

        @block.sync
        def _(sync):
            for it in range(NIT):
                j, sb = it // NSB, it % NSB
                b = it % 2
                if it >= 2:
                    sync.wait_ge(s_cmb, it - 1)
                    # idx[b] consumed once gather(it-2) completed
                    sync.wait_ge(s_gat[b], 16 * GSPLIT * (it // 2))
                sync.dma_start(idxs[:, b, :], idx[j, sb]).then_inc(s_idx[b], 16)
                sync.dma_start(w8t[:, b, :], w8[j, sb]).then_inc(s_w8[b], 16)
            # final output DMA after relu (contiguous, same layout)
            sync.wait_ge(s_relu, 1)
            sync.dma_start(out[:], outt[:]).then_inc(s_out, 16)
            sync.wait_ge(s_out, 16)

        @block.vector
        def _(vector):
            sv = 0

            def emit(inst):
                nonlocal sv
                sv += 1
                inst.then_inc(s_v, 1)

            def barrier():
                vector.wait_ge(s_v, sv)

            emit(vector.memset(outt[:].rearrange("P a b c -> P (a b c)"), 0.0))
            barrier()
            for it in range(NIT):
                j, sb = it // NSB, it % NSB
                b = it % 2
                vector.wait_ge(s_gat[b], 16 * GSPLIT * (it // 2 + 1))
                vector.wait_ge(s_w8[b], 16 * (it // 2 + 1))
                # plane chunks [p,nb | c,r8] * w8p [p,nb,r8] bcast c
                in0 = dst[:, b, 0:3 * NB, :].rearrange(
                    "P (p nb) (c r) -> P p nb c r", p=3, c=16)
                in1 = w8t[:, b, 0:384].rearrange(
                    "P (p nb r) -> P p nb r", p=3, nb=NB
                ).unsqueeze(3).broadcast_to([128, 3, NB, 16, 8])
                wv = wprod[:].rearrange(
                    "P (p nb c r) -> P p nb c r", p=3, nb=NB, c=16)
                emit(vector.tensor_tensor(wv, in0, in1, mybir.AluOpType.mult))
                # pair chunks [nb | l,c,r4] * wpair [nb,l,r4] bcast c
                q0 = dst[:, b, 3 * NB:4 * NB, :].rearrange(
                    "P nb (l c r) -> P nb l c r", l=2, c=16)
                q1 = w8t[:, b, 384:512].rearrange(
                    "P (nb l r) -> P nb l r", nb=NB, l=2
                ).unsqueeze(3).broadcast_to([128, NB, 2, 16, 4])
                qv = wq[:].rearrange("P (nb l c r) -> P nb l c r", nb=NB, l=2, c=16)
                emit(vector.tensor_tensor(qv, q0, q1, mybir.AluOpType.mult))
                # line2 chunks [nb | c,r8] * wl2 [nb,r8] bcast c
                z0 = dst[:, b, 4 * NB:5 * NB, :].rearrange(
                    "P nb (c r) -> P nb c r", c=16)
                z1 = w8t[:, b, 512:640].rearrange(
                    "P (nb r) -> P nb r", nb=NB
                ).unsqueeze(2).broadcast_to([128, NB, 16, 8])
                zv = w2[:].rearrange("P (nb c r) -> P nb c r", nb=NB, c=16)
                emit(vector.tensor_tensor(zv, z0, z1, mybir.AluOpType.mult))
                barrier()
                # add trees: planes 8->4->2->1, pair 4->2->1, line2 8->4->2->1
                w3 = wprod[:].rearrange("P (m r) -> P m r", r=8)
                t1v = t1[:].rearrange("P (m r) -> P m r", r=4)
                emit(vector.tensor_tensor(t1v, w3[:, :, 0:4], w3[:, :, 4:8],
                                          mybir.AluOpType.add))
                wqv = wq[:].rearrange("P (m r) -> P m r", r=4)
                wq1v = wq1[:].rearrange("P (m r) -> P m r", r=2)
                emit(vector.tensor_tensor(wq1v, wqv[:, :, 0:2], wqv[:, :, 2:4],
                                          mybir.AluOpType.add))
                w2v = w2[:].rearrange("P (m r) -> P m r", r=8)
                w21v = w21[:].rearrange("P (m r) -> P m r", r=4)
                emit(vector.tensor_tensor(w21v, w2v[:, :, 0:4], w2v[:, :, 4:8],
                                          mybir.AluOpType.add))
                barrier()
                t2v = t2[:].rearrange("P (m r) -> P m r", r=2)
                emit(vector.tensor_tensor(t2v, t1v[:, :, 0:2], t1v[:, :, 2:4],
                                          mybir.AluOpType.add))
                emit(vector.tensor_tensor(lf01[:], wq1v[:, :, 0], wq1v[:, :, 1],
                                          mybir.AluOpType.add))
                w22v = w22[:].rearrange("P (m r) -> P m r", r=2)
                emit(vector.tensor_tensor(w22v, w21v[:, :, 0:2], w21v[:, :, 2:4],
                                          mybir.AluOpType.add))
                barrier()
                emit(vector.tensor_tensor(pf[:], t2v[:, :, 0], t2v[:, :, 1],
                                          mybir.AluOpType.add))
                emit(vector.tensor_tensor(lf2[:], w22v[:, :, 0], w22v[:, :, 1],
                                          mybir.AluOpType.add))
                barrier()
                # prod = pf * lf ; lf01 layout [nb,l,c] -> [l,nb,c]
                lq = lf01[:].rearrange("P (nb l c) -> P l nb c", nb=NB, l=2)
                p01 = pf[:, 0:512].rearrange("P (p nb c) -> P p nb c", p=2, c=16)
                emit(vector.tensor_tensor(
                    prod[:, 0:512].rearrange("P (p nb c) -> P p nb c", p=2, c=16),
                    p01, lq, mybir.AluOpType.mult))
                emit(vector.tensor_tensor(prod[:, 512:768], pf[:, 512:768],
                                          lf2[:], mybir.AluOpType.mult))
                barrier()
                pv = prod[:].rearrange("P (p nb c) -> P nb p c", p=3, c=16)
                vector.tensor_reduce(
                    outt[:, sb, :, j], pv, mybir.AxisListType.XY,
                    mybir.AluOpType.add,
                ).then_inc(s_cmb, 1)
            vector.wait_ge(s_cmb, NIT)
            of = outt[:].rearrange("P a b c -> P (a b c)")
            vector.tensor_scalar_max(of, of, 0.0).then_inc(s_relu, 1)

    nc.compile()
    return nc


# ---------------- entry point ----------------
def kernel(xyz, transforms, plane0, plane1, plane2, line0, line1, line2):
    planes = [np.asarray(plane0), np.asarray(plane1), np.asarray(plane2)]
    lines = [np.asarray(line0), np.asarray(line1), np.asarray(line2)]
    tab = _build_tables(planes, lines)
    idx5, w8p, wpair, wl2 = _coords_weights(np.asarray(xyz), np.asarray(transforms))

    in_maps = []
    for k in range(N_CORES):
        s = slice(k * NPTS, (k + 1) * NPTS)
        idx_d, w8_d = _pack_core(idx5[s], w8p[s], wpair[s], wl2[s])
        in_maps.append({"tab": tab, "idx": idx_d, "w8": w8_d})

    if "nc" not in _CACHE:
        _CACHE["nc"] = _build_bass()
    nc = _CACHE["nc"]

    _CACHE["in_maps"] = in_maps
    res = run_bass_kernel_spmd(nc, in_maps, core_ids=list(range(N_CORES)))
    outs = []
    for r in res.results:
        o = np.asarray(r["out"]).reshape(128, NSB, NB, J)
        outs.append(o.transpose(1, 2, 0, 3).reshape(NPTS, J))
    return np.concatenate(outs, axis=0).astype(np.float32)


if __name__ == "__main__":
    rng = np.random.default_rng(0)
    xyz = (rng.random((N_TOTAL, 3), np.float32) * 2 - 1).astype(np.float32)
    tr = (np.eye(4, dtype=np.float32)[None]
          + 0.05 * rng.standard_normal((J, 4, 4)).astype(np.float32))
    pl = [(0.032 * rng.standard_normal((J, C, G, G))).astype(np.float32) for _ in range(3)]
    ln = [(0.032 * rng.standard_normal((J, C, G))).astype(np.float32) for _ in range(3)]
    o = kernel(xyz, tr, pl[0], pl[1], pl[2], ln[0], ln[1], ln[2])
    print(o.shape, o.dtype, float(o.max()))



# revision 5
# speedup vs baseline: 8.3020x; 8.3020x over previous
"""Trainium2 Bass kernel for nn_BWCaster_86337432584570 (embedding_lookup), v3.

sigma[n,j] = relu( sum_p sum_c bilinear(plane_p[j])[c] * linear(line_p[j])[c] )

v3 design: 3 fused 256B gather rows per (point, joint) — each plane's row
carries a line's 2 exact taps in its padding (the line coordinate equals an
exact row/col index of some plane):
  row0 (y=|g1|, x=|g0|): P0[c,2y,2x] (64) | L2[c, x:x+2] (32) | pad
  row1 (y=|g2|, x=|g0|): P1[c,2y,2x]      | L0[c, y:y+2]      | pad
  row2 (y=|g2|, x=|g1|): P2[c,2y,2x]      | L1[c, x:x+2]      | pad
Gathers run on 4 SWDGE queues (6 calls/iter, 12 slots per queue) — the
per-queue descriptor ring is the bottleneck (~7-9 ns/desc/queue).
"""
import sys
import numpy as np
import ml_dtypes

sys.path.insert(0, "/opt/trn_rl_repo")

import concourse.bass as bass
import concourse.bacc as bacc
import concourse.mybir as mybir
from concourse.bass_utils import run_bass_kernel_spmd
from concourse.library_config import mlp

# ---------------- problem constants (hardcoded) ----------------
N_TOTAL, J, C, G = 262144, 24, 16, 128
N_CORES = 8
NPTS = N_TOTAL // N_CORES          # 32768 points per core
SB = 2048                          # points per superblock
NSB = NPTS // SB                   # 16
NB = SB // 128                     # 16 sub-blocks of 128 points
NSLOT = 3 * NB                     # 48 gather slots per (j, superblock)
NIDX = NSLOT * 128                 # 6144 indices per (j, superblock)
DEPTH = 4                          # pipeline depth (gather buffers)
NROWS = 127 * 127                  # 16129 rows per (j, p) table
WLEN = 3 * NB * 4 + 3 * NB * 2     # 288 weights per partition-point
# gather sub-calls: (p, slot_lo, slot_hi, queue)
GCALLS = [(0, 0, 12, 0), (0, 12, 16, 1),
          (1, 0, 8, 1), (1, 8, 16, 2),
          (2, 0, 4, 2), (2, 4, 16, 3)]
BF16 = mybir.dt.bfloat16
F32 = mybir.dt.float32
I16 = mybir.dt.int16

_CACHE = {}


# ---------------- host-side prep ----------------
def _build_tables(planes, lines):
    """tab [3, J, NROWS, 128] bf16; row (y,x) of table p:
    [16c x (2y,2x)] plane corners | [16c x 2] line taps | 32 pad."""
    tab = np.zeros((3, J, NROWS, 128), dtype=np.float32)
    for p in range(3):
        sw = np.lib.stride_tricks.sliding_window_view(planes[p], (2, 2), axis=(2, 3))
        # [J, C, 127, 127, 2, 2] -> [J, y, x, C, 2, 2]
        tab[p, :, :, 0:64] = sw.transpose(0, 2, 3, 1, 4, 5).reshape(J, NROWS, 64)
    # line tails (2 exact taps, broadcast over the unused row coordinate)
    l2 = np.lib.stride_tricks.sliding_window_view(lines[2], 2, axis=2)  # [J,C,127,2]
    t0 = np.broadcast_to(l2.transpose(0, 2, 1, 3)[:, None, :, :, :],
                         (J, 127, 127, C, 2))                  # bcast over y
    tab[0, :, :, 64:96] = t0.reshape(J, NROWS, 32)
    l0 = np.lib.stride_tricks.sliding_window_view(lines[0], 2, axis=2)
    t1 = np.broadcast_to(l0.transpose(0, 2, 1, 3)[:, :, None, :, :],
                         (J, 127, 127, C, 2))                  # bcast over x
    tab[1, :, :, 64:96] = t1.reshape(J, NROWS, 32)
    l1 = np.lib.stride_tricks.sliding_window_view(lines[1], 2, axis=2)
    t2 = np.broadcast_to(l1.transpose(0, 2, 1, 3)[:, None, :, :, :],
                         (J, 127, 127, C, 2))                  # bcast over y
    tab[2, :, :, 64:96] = t2.reshape(J, NROWS, 32)
    return tab.astype(ml_dtypes.bfloat16)


def _coords_weights(xyz, transforms):
    """Returns idx3 [N,J,3] int16, wp [N,J,3,4] f32, wl [N,J,3,2] f32.
    wl is in ROW order (row0=L2, row1=L0, row2=L1)."""
    N = xyz.shape[0]
    xyzh = np.concatenate([xyz, np.ones((N, 1), np.float32)], axis=1)
    pts = np.einsum('jab,nb->nja', transforms[:, :3, :].astype(np.float32), xyzh)
    coord = (pts * np.float32(2.0 / 3.0) + np.float32(1.0)) * np.float32(0.5 * (G - 1))
    c0 = np.floor(coord).astype(np.int32)          # [N,J,3] per-axis cell
    fr = (coord - c0).astype(np.float32)
    c0c = np.clip(c0, 0, 126)                      # safety; margin makes this a no-op

    # (y_axis, x_axis) per plane row: row0=(1,0), row1=(2,0), row2=(2,1)
    YX = [(1, 0), (2, 0), (2, 1)]
    idx3 = np.empty((N, J, 3), np.int16)
    wp = np.empty((N, J, 3, 4), np.float32)
    wl = np.empty((N, J, 3, 2), np.float32)
    for p, (ya, xa) in enumerate(YX):
        y0, x0 = c0c[:, :, ya], c0c[:, :, xa]
        fy, fx = fr[:, :, ya], fr[:, :, xa]
        idx3[:, :, p] = (y0 * 127 + x0).astype(np.int16)
        wy0, wx0 = 1.0 - fy, 1.0 - fx
        wp[:, :, p, 0] = wy0 * wx0
        wp[:, :, p, 1] = wy0 * fx
        wp[:, :, p, 2] = fy * wx0
        wp[:, :, p, 3] = fy * fx
    # line taps: row0 tail = L2 @ g0 (axis 0), row1 = L0 @ g2, row2 = L1 @ g1
    for p, ax in enumerate([0, 2, 1]):
        f = fr[:, :, ax]
        wl[:, :, p, 0] = 1.0 - f
        wl[:, :, p, 1] = f
    return idx3, wp, wl


def _pack_core(idx3, wp, wl):
    """idx3 [NPTS,J,3] -> idx dram [J,NSB,128,NIDX//16] int16 (wrapped+replicated)
    wp/wl -> w dram [J,NSB,128,WLEN] bf16: [3p,16nb,4 | 3row,16nb,2]."""
    u = idx3.reshape(NSB, NB, 128, J, 3)
    arr = u.transpose(3, 0, 4, 1, 2).reshape(J, NSB, NSLOT * 128)
    wrapped = arr.reshape(J, NSB, NIDX // 16, 16).transpose(0, 1, 3, 2)
    idx_dram = np.broadcast_to(wrapped[:, :, None, :, :], (J, NSB, 8, 16, NIDX // 16))
    idx_dram = np.ascontiguousarray(idx_dram).reshape(J, NSB, 128, NIDX // 16)

    a = wp.reshape(NSB, NB, 128, J, 3, 4).transpose(3, 0, 2, 4, 1, 5)  # [j,sb,np,p,nb,4]
    b = wl.reshape(NSB, NB, 128, J, 3, 2).transpose(3, 0, 2, 4, 1, 5)  # [j,sb,np,r,nb,2]
    w_dram = np.concatenate([
        np.ascontiguousarray(a).reshape(J, NSB, 128, 192),
        np.ascontiguousarray(b).reshape(J, NSB, 128, 96),
    ], axis=3).astype(ml_dtypes.bfloat16)
    return idx_dram, w_dram


# ---------------- device kernel ----------------
def _build_bass(nit_lim=None):
    """nit_lim > 384 wraps (for slope timing); real workload is NIT=384."""
    nc = bacc.Bacc("TRN2", num_swdge_queues=4)
    tab = nc.dram_tensor("tab", [3, J, NROWS, 128], BF16, kind="ExternalInput")
    idx = nc.dram_tensor("idx", [J, NSB, 128, NIDX // 16], I16, kind="ExternalInput")
    w8 = nc.dram_tensor("w8", [J, NSB, 128, WLEN], BF16, kind="ExternalInput")
    # device-native layout [np, sb, nb, j]; host transposes to [NPTS, J]
    out = nc.dram_tensor("out", [128, NSB, NB, J], F32, kind="ExternalOutput")

    NIT = J * NSB  # 384 iterations, j outer / sb inner
    if nit_lim is not None:
        NIT = nit_lim
    D = DEPTH
    NCALL = len(GCALLS)

    from contextlib import ExitStack
    with ExitStack() as ctx:
        dst = ctx.enter_context(nc.sbuf_tensor("dst", [128, D, NSLOT, 128], BF16))
        idxs = ctx.enter_context(nc.sbuf_tensor("idxs", [128, D, NIDX // 16], I16))
        w8t = ctx.enter_context(nc.sbuf_tensor("w8t", [128, D, WLEN], BF16))
        wprod = ctx.enter_context(nc.sbuf_tensor("wprod", [128, 3072], BF16))
        t1 = ctx.enter_context(nc.sbuf_tensor("t1", [128, 1536], BF16))
        pf = ctx.enter_context(nc.sbuf_tensor("pf", [128, 768], F32))
        lft = ctx.enter_context(nc.sbuf_tensor("lft", [128, 1536], BF16))
        lf = ctx.enter_context(nc.sbuf_tensor("lf", [128, 768], F32))
        prod = ctx.enter_context(nc.sbuf_tensor("prod", [128, NB, 3, 16], F32))
        outt = ctx.enter_context(nc.sbuf_tensor("outt", [128, NSB, NB, J], F32))
        s_gat = [ctx.enter_context(nc.semaphore(f"s_gat{i}")) for i in range(D)]
        s_idx = [ctx.enter_context(nc.semaphore(f"s_idx{i}")) for i in range(D)]
        s_w8 = [ctx.enter_context(nc.semaphore(f"s_w8{i}")) for i in range(D)]
        s_cmb = ctx.enter_context(nc.semaphore("s_cmb"))
        s_relu = ctx.enter_context(nc.semaphore("s_relu"))
        s_out = ctx.enter_context(nc.semaphore("s_out"))
        s_v = ctx.enter_context(nc.semaphore("s_v"))
        block = ctx.enter_context(nc.Block())

        @block.gpsimd
        def _(gpsimd):
            gpsimd.load_library(mlp)
            for it in range(NIT):
                e = it % 384
                j = e // NSB
                b = it % D
                if it >= D:
                    # dst[b] free once compute(it-D) done
                    gpsimd.wait_ge(s_cmb, it - (D - 1))
                gpsimd.wait_ge(s_idx[b], 16 * (it // D + 1))
                for (p, lo, hi, q) in GCALLS:
                    s0 = p * NB + lo
                    s1 = p * NB + hi
                    n = (hi - lo) * 128
                    gpsimd.dma_gather(
                        dst[:, b, s0:s1, :], tab[p, j],
                        idxs[:, b, s0 * 8:s1 * 8],
                        n, n, 128, single_packet=False, queue_num=q,
                    ).then_inc(s_gat[b], 16)

        @block.sync
        def _(sync):
            for it in range(NIT):
                e = it % 384
                j, sb = e // NSB, e % NSB
                b = it % D
                if it >= D:
                    # w8t[b] free once compute(it-D) done
                    sync.wait_ge(s_cmb, it - (D - 1))
                    # idxs[b] consumed once gather(it-D) completed
                    sync.wait_ge(s_gat[b], 16 * NCALL * (it // D))
                sync.dma_start(idxs[:, b, :], idx[j, sb]).then_inc(s_idx[b], 16)
                sync.dma_start(w8t[:, b, :], w8[j, sb]).then_inc(s_w8[b], 16)
            # final output DMA after relu (contiguous, same layout)
            sync.wait_ge(s_relu, 1)
            sync.dma_start(out[:], outt[:]).then_inc(s_out, 16)
            sync.wait_ge(s_out, 16)

        @block.vector
        def _(vector):
            sv = 0

            def emit(inst):
                nonlocal sv
                sv += 1
                inst.then_inc(s_v, 1)

            def barrier():
                vector.wait_ge(s_v, sv)

            emit(vector.memset(outt[:].rearrange("P a b c -> P (a b c)"), 0.0))
            barrier()
            for it in range(NIT):
                e = it % 384
                j, sb = e // NSB, e % NSB
                b = it % D
                vector.wait_ge(s_gat[b], 16 * NCALL * (it // D + 1))
                vector.wait_ge(s_w8[b], 16 * (it // D + 1))
                # plane corners [3p,16nb | 16c,4yx] * wp [3p,16nb,4] bcast c
                in0 = dst[:, b, :, 0:64].rearrange(
                    "P (p nb) (c r) -> P p nb c r", p=3, c=16)
                in1 = w8t[:, b, 0:192].rearrange(
                    "P (p nb r) -> P p nb r", p=3, nb=NB
                ).unsqueeze(3).broadcast_to([128, 3, NB, 16, 4])
                wv = wprod[:].rearrange(
                    "P (p nb c r) -> P p nb c r", p=3, nb=NB, c=16)
                emit(vector.tensor_tensor(wv, in0, in1, mybir.AluOpType.mult))
                # line taps [3row,16nb | 16c,2] * wl [3row,16nb,2] bcast c
                li0 = dst[:, b, :, 64:96].rearrange(
                    "P (p nb) (c r) -> P p nb c r", p=3, c=16)
                li1 = w8t[:, b, 192:288].rearrange(
                    "P (p nb r) -> P p nb r", p=3, nb=NB
                ).unsqueeze(3).broadcast_to([128, 3, NB, 16, 2])
                lv = lft[:].rearrange(
                    "P (p nb c r) -> P p nb c r", p=3, nb=NB, c=16)
                emit(vector.tensor_tensor(lv, li0, li1, mybir.AluOpType.mult))
                barrier()
                # plane tree 4 -> 2 -> 1 ; line tree 2 -> 1
                w3 = wprod[:].rearrange("P (m r) -> P m r", r=4)
                t1v = t1[:].rearrange("P (m r) -> P m r", r=2)
                emit(vector.tensor_tensor(t1v, w3[:, :, 0:2], w3[:, :, 2:4],
                                          mybir.AluOpType.add))
                l3 = lft[:].rearrange("P (m r) -> P m r", r=2)
                emit(vector.tensor_tensor(lf[:], l3[:, :, 0], l3[:, :, 1],
                                          mybir.AluOpType.add))
                barrier()
                emit(vector.tensor_tensor(pf[:], t1v[:, :, 0], t1v[:, :, 1],
                                          mybir.AluOpType.add))
                barrier()
                # prod[nb, p, c] = pf[p, nb, c] * lf[rowof(p), nb, c]
                pfv = pf[:].rearrange("P (p nb c) -> P p nb c", p=3, c=16)
                lfv = lf[:].rearrange("P (p nb c) -> P p nb c", p=3, c=16)
                for p, row in enumerate([1, 2, 0]):
                    emit(vector.tensor_tensor(
                        prod[:, :, p, :], pfv[:, p], lfv[:, row],
                        mybir.AluOpType.mult))
                barrier()
                vector.tensor_reduce(
                    outt[:, sb, :, j],
                    prod[:].rearrange("P nb p c -> P nb (p c)"),
                    mybir.AxisListType.X, mybir.AluOpType.add,
                ).then_inc(s_cmb, 1)
            vector.wait_ge(s_cmb, NIT)
            of = outt[:].rearrange("P a b c -> P (a b c)")
            vector.tensor_scalar_max(of, of, 0.0).then_inc(s_relu, 1)

    nc.compile()
    return nc


# ---------------- entry point ----------------
def prepare_in_maps(inputs):
    planes = [np.asarray(inputs[f"plane{i}"]) for i in range(3)]
    lines = [np.asarray(inputs[f"line{i}"]) for i in range(3)]
    tab = _build_tables(planes, lines)
    idx3, wp, wl = _coords_weights(
        np.asarray(inputs["xyz"]), np.asarray(inputs["transforms"]))
    in_maps = []
    for k in range(N_CORES):
        s = slice(k * NPTS, (k + 1) * NPTS)
        idx_d, w_d = _pack_core(idx3[s], wp[s], wl[s])
        in_maps.append({"tab": tab, "idx": idx_d, "w8": w_d})
    return in_maps


def kernel(xyz, transforms, plane0, plane1, plane2, line0, line1, line2):
    in_maps = prepare_in_maps(dict(
        xyz=xyz, transforms=transforms, plane0=plane0, plane1=plane1,
        plane2=plane2, line0=line0, line1=line1, line2=line2))

    if "nc" not in _CACHE:
        _CACHE["nc"] = _build_bass()
    nc = _CACHE["nc"]

    _CACHE["in_maps"] = in_maps
    res = run_bass_kernel_spmd(nc, in_maps, core_ids=list(range(N_CORES)))
    outs = []
    for r in res.results:
        o = np.asarray(r["out"]).reshape(128, NSB, NB, J)
        outs.append(o.transpose(1, 2, 0, 3).reshape(NPTS, J))
    return np.concatenate(outs, axis=0).astype(np.float32)


if __name__ == "__main__":
    rng = np.random.default_rng(0)
    xyz = (rng.random((N_TOTAL, 3), np.float32) * 2 - 1).astype(np.float32)
    tr = (np.eye(4, dtype=np.float32)[None]
          + 0.05 * rng.standard_normal((J, 4, 4)).astype(np.float32))
    pl = [(0.032 * rng.standard_normal((J, C, G, G))).astype(np.float32) for _ in range(3)]
    ln = [(0.032 * rng.standard_normal((J, C, G))).astype(np.float32) for _ in range(3)]
    o = kernel(xyz, tr, pl[0], pl[1], pl[2], ln[0], ln[1], ln[2])
    print(o.shape, o.dtype, float(o.max()))
